# revision 4
# baseline (speedup 1.0000x reference)
"""Grouped MLP (8-expert SwiGLU) Trainium2 Bass kernel.

Sharding: expert-parallel, one group per NeuronCore (8 cores).
Token t belongs to group t % 8, so core n gets x[n::8] (4096 tokens),
its expert's gate/up/down weights, and produces out[n::8].

Device-side layout trick: x is transposed on the host during sharding, so
the device receives xT [d_in, tokens] and never needs an on-chip transpose.
gate/up matmuls produce hidden in [d_hid, tokens] layout, which is exactly
the lhsT layout the down-projection needs; all weights stay in their
natural [k, n] layouts.

All matmul operands are bf16 (converted on the host): the PE runs bf16 at
the same 1 row/cycle as fp32r, but bf16 halves HBM traffic and SBUF
footprint, which lets all three weight matrices stay resident in SBUF for
the whole kernel (no per-token-block reload), and enables the FWL fast
weight-load path (fp32 matmuls force the FP32-HIGH mode that disables it).
Accumulation stays fp32 in PSUM and the output is written fp32.
"""

import sys

if "/opt/trn_rl_repo" not in sys.path:
    sys.path.insert(0, "/opt/trn_rl_repo")

import numpy as np

import concourse.bass as bass  # noqa: F401  (registers bass machinery)
import concourse.tile as tile
from concourse import bacc, mybir
from concourse.bass_utils import run_bass_kernel_spmd

P = 128
T = 4096   # tokens per core (per group)
K = 1024   # d_in
H = 2048   # d_hid
O = 1024   # d_out
N_CORES = 8

F32 = mybir.dt.float32
BF16 = mybir.dt.bfloat16

# Tiling knobs
TB = 1024           # token block
WCH = 128           # gate/up weight DMA chunk width along hidden dim
MMF = 512           # matmul moving free dim (one fp32 PSUM bank)

KO = K // P         # 8  k-subtiles
HO = H // P         # 16 h-subtiles
NTB = T // TB       # token blocks
NT = TB // MMF      # 512-wide t-tiles per token block
NWC = H // WCH      # weight DMA chunks
NO = O // MMF       # 512-wide o-tiles

_CACHED_NC = None


def _build_nc():
    from contextlib import ExitStack

    nc = bacc.Bacc(None, target_bir_lowering=False)
    xt = nc.dram_tensor("xt", [K, T], BF16, kind="ExternalInput")
    wg = nc.dram_tensor("wg", [K, H], BF16, kind="ExternalInput")
    wu = nc.dram_tensor("wu", [K, H], BF16, kind="ExternalInput")
    wd = nc.dram_tensor("wd", [H, O], BF16, kind="ExternalInput")
    out = nc.dram_tensor("out", [T, O], F32, kind="ExternalOutput")

    silu_fn = mybir.ActivationFunctionType.Silu

    with tile.TileContext(nc) as tc, ExitStack() as ctx:
        wpool = ctx.enter_context(tc.tile_pool(name="wpool", bufs=1))
        xpool = ctx.enter_context(tc.tile_pool(name="xpool", bufs=2))
        hpool = ctx.enter_context(tc.tile_pool(name="hpool", bufs=1))
        spool = ctx.enter_context(tc.tile_pool(name="spool", bufs=2))
        opool = ctx.enter_context(tc.tile_pool(name="opool", bufs=2))
        zpool = ctx.enter_context(tc.tile_pool(name="zpool", bufs=1))
        ps12 = ctx.enter_context(tc.tile_pool(name="ps12", bufs=2, space="PSUM"))
        ps3 = ctx.enter_context(tc.tile_pool(name="ps3", bufs=3, space="PSUM"))
        psz = ctx.enter_context(tc.tile_pool(name="psz", bufs=1, space="PSUM"))

        # All weights resident for the whole kernel (bf16: 96 KiB/partition).
        wg_sb = wpool.tile([P, KO, H], BF16)
        wu_sb = wpool.tile([P, KO, H], BF16)
        wd_sb = wpool.tile([P, HO, O], BF16)

        # Warm-up: the PE clock gate (HAM) only opens to 2.4 GHz after ~3.4us
        # of sustained activity.  Burn dummy matmuls on a zeroed scratch tile
        # while the first weight/x DMAs are still in flight, so the real
        # matmul stream starts at full clock instead of paying the ramp.
        zero_sb = zpool.tile([P, MMF], BF16)
        nc.gpsimd.memset(zero_sb[:], 0)
        zero_ps = psz.tile([P, MMF], F32)
        for _ in range(7):
            nc.tensor.matmul(
                zero_ps[:], zero_sb[:, 0:P], zero_sb[:], start=True, stop=True
            )

        xt_tiles = {}

        def load_xt(tb):
            # xt rides the gpsimd DMA queue: descriptor-issue for the 8
            # slices would otherwise serialize ahead of the weight-chunk
            # issues on the sync queue and starve the first gate chains.
            t = xpool.tile([P, KO, TB], BF16, tag="xt", name=f"xt_sb{tb}")
            for ko in range(KO):
                nc.gpsimd.dma_start(
                    t[:, ko, :],
                    xt[ko * P : (ko + 1) * P, tb * TB : (tb + 1) * TB],
                )
            xt_tiles[tb] = t

        # Startup DMA order: the very first gate-chain matmul needs only
        # wg[k-block 0, h-cols 0:128] (32 KB) + the first xt k-slice, so ship
        # that fragment first; wd is only read ~100us in (block-0 down
        # projection), so it loads last.
        nc.sync.dma_start(wg_sb[:, 0, 0:WCH], wg[0:P, 0:WCH])
        load_xt(0)
        nc.sync.dma_start(
            wg_sb[:, 1:, 0:WCH],
            wg[P:, 0:WCH].rearrange("(ko p) h -> p ko h", p=P),
        )
        nc.sync.dma_start(
            wu_sb[:, :, 0:WCH],
            wu[:, 0:WCH].rearrange("(ko p) h -> p ko h", p=P),
        )
        for c in range(1, NWC):
            csl = slice(c * WCH, (c + 1) * WCH)
            nc.sync.dma_start(
                wg_sb[:, :, csl],
                wg[:, csl].rearrange("(ko p) h -> p ko h", p=P),
            )
            nc.sync.dma_start(
                wu_sb[:, :, csl],
                wu[:, csl].rearrange("(ko p) h -> p ko h", p=P),
            )
        for ho in range(HO):
            nc.sync.dma_start(wd_sb[:, ho, :], wd[ho * P : (ho + 1) * P, :])

        for tb in range(NTB):
            if tb + 1 < NTB:
                load_xt(tb + 1)
            xt_sb = xt_tiles.pop(tb)
            hid_sb = hpool.tile([P, HO, TB], BF16, tag="hid")

            for h in range(HO):
                for th in range(NT):
                    tsl = slice(th * MMF, (th + 1) * MMF)
                    gate_ps = ps12.tile([P, MMF], F32, tag="gate")
                    for ko in range(KO):
                        nc.tensor.matmul(
                            gate_ps[:],
                            wg_sb[:, ko, h * P : (h + 1) * P],
                            xt_sb[:, ko, tsl],
                            start=(ko == 0),
                            stop=(ko == KO - 1),
                        )
                    up_ps = ps12.tile([P, MMF], F32, tag="up")
                    for ko in range(KO):
                        nc.tensor.matmul(
                            up_ps[:],
                            wu_sb[:, ko, h * P : (h + 1) * P],
                            xt_sb[:, ko, tsl],
                            start=(ko == 0),
                            stop=(ko == KO - 1),
                        )
                    silu_sb = spool.tile([P, MMF], F32, tag="silu")
                    nc.scalar.activation(silu_sb[:], gate_ps[:], silu_fn)
                    nc.vector.tensor_mul(
                        hid_sb[:, h, tsl], silu_sb[:], up_ps[:]
                    )

            # Down projection for this token block.
            for ti in range(TB // P):
                for oi in range(NO):
                    osl = slice(oi * MMF, (oi + 1) * MMF)
                    out_ps = ps3.tile([P, MMF], F32, tag="outp")
                    for ho in range(HO):
                        nc.tensor.matmul(
                            out_ps[:],
                            hid_sb[:, ho, ti * P : (ti + 1) * P],
                            wd_sb[:, ho, osl],
                            start=(ho == 0),
                            stop=(ho == HO - 1),
                        )
                    # Copy + out-DMA both on the scalar queue (idle during the
                    # down phase): same-queue ordering needs no cross-engine
                    # semaphore, and vector stays free for the SwiGLU muls.
                    ob = opool.tile([P, MMF], F32, tag="ob")
                    nc.scalar.activation(
                        ob[:], out_ps[:], mybir.ActivationFunctionType.Copy
                    )
                    nc.scalar.dma_start(
                        out[tb * TB + ti * P : tb * TB + (ti + 1) * P, osl],
                        ob[:],
                    )

    nc.compile()
    return nc


def _get_nc():
    global _CACHED_NC
    if _CACHED_NC is None:
        _CACHED_NC = _build_nc()
    return _CACHED_NC


def _make_in_maps(x, gate_weight, up_weight, down_weight, n):
    import ml_dtypes

    bf16 = ml_dtypes.bfloat16
    in_maps = []
    for g in range(n):
        in_maps.append(
            {
                "xt": np.ascontiguousarray(x[g::n].T).astype(bf16),
                "wg": np.ascontiguousarray(gate_weight[g]).astype(bf16),
                "wu": np.ascontiguousarray(up_weight[g]).astype(bf16),
                "wd": np.ascontiguousarray(down_weight[g]).astype(bf16),
            }
        )
    return in_maps


def _run_spmd(in_maps, **kwargs):
    nc = _get_nc()
    return run_bass_kernel_spmd(nc, in_maps, core_ids=list(range(N_CORES)), **kwargs)


def kernel(x, gate_weight, up_weight, down_weight, num_groups=8):
    n = int(num_groups)
    x = np.asarray(x, dtype=np.float32)
    gate_weight = np.asarray(gate_weight, dtype=np.float32)
    up_weight = np.asarray(up_weight, dtype=np.float32)
    down_weight = np.asarray(down_weight, dtype=np.float32)

    assert n == N_CORES, f"expected {N_CORES} groups, got {n}"
    assert x.shape == (T * N_CORES, K), x.shape
    assert gate_weight.shape == (n, K, H), gate_weight.shape
    assert up_weight.shape == (n, K, H), up_weight.shape
    assert down_weight.shape == (n, H, O), down_weight.shape

    in_maps = _make_in_maps(x, gate_weight, up_weight, down_weight, n)
    res = _run_spmd(in_maps)

    out = np.empty((x.shape[0], O), dtype=np.float32)
    for g in range(n):
        out[g::n] = res.results[g]["out"]
    return out


# revision 6
# speedup vs baseline: 1.0038x; 1.0038x over previous
"""Grouped MLP (8-expert SwiGLU) Trainium2 Bass kernel.

Sharding: expert-parallel, one group per NeuronCore (8 cores).
Token t belongs to group t % 8, so core n gets x[n::8] (4096 tokens),
its expert's gate/up/down weights, and produces out[n::8].

Device-side layout trick: x is transposed on the host during sharding, so
the device receives xT [d_in, tokens] and never needs an on-chip transpose.
gate/up matmuls produce hidden in [d_hid, tokens] layout, which is exactly
the lhsT layout the down-projection needs; all weights stay in their
natural [k, n] layouts.

All matmul operands are bf16 (converted on the host): the PE runs bf16 at
the same 1 row/cycle as fp32r, but bf16 halves HBM traffic and SBUF
footprint, which lets all three weight matrices stay resident in SBUF for
the whole kernel (no per-token-block reload), and enables the FWL fast
weight-load path (fp32 matmuls force the FP32-HIGH mode that disables it).
Accumulation stays fp32 in PSUM and the output is written fp32.
"""

import sys

if "/opt/trn_rl_repo" not in sys.path:
    sys.path.insert(0, "/opt/trn_rl_repo")

import numpy as np

import concourse.bass as bass  # noqa: F401  (registers bass machinery)
import concourse.tile as tile
from concourse import bacc, mybir
from concourse.bass_utils import run_bass_kernel_spmd

P = 128
T = 4096   # tokens per core (per group)
K = 1024   # d_in
H = 2048   # d_hid
O = 1024   # d_out
N_CORES = 8

F32 = mybir.dt.float32
BF16 = mybir.dt.bfloat16

# Tiling knobs
TB = 1024           # token block
WCH = 128           # gate/up weight DMA chunk width along hidden dim
MMF = 512           # matmul moving free dim (one fp32 PSUM bank)

KO = K // P         # 8  k-subtiles
HO = H // P         # 16 h-subtiles
NTB = T // TB       # token blocks
NT = TB // MMF      # 512-wide t-tiles per token block
NWC = H // WCH      # weight DMA chunks
NO = O // MMF       # 512-wide o-tiles

_CACHED_NC = None


def _build_nc():
    from contextlib import ExitStack

    nc = bacc.Bacc(None, target_bir_lowering=False)
    xt = nc.dram_tensor("xt", [K, T], BF16, kind="ExternalInput")
    wg = nc.dram_tensor("wg", [K, H], BF16, kind="ExternalInput")
    wu = nc.dram_tensor("wu", [K, H], BF16, kind="ExternalInput")
    wd = nc.dram_tensor("wd", [H, O], BF16, kind="ExternalInput")
    out = nc.dram_tensor("out", [T, O], F32, kind="ExternalOutput")

    silu_fn = mybir.ActivationFunctionType.Silu

    with tile.TileContext(nc) as tc, ExitStack() as ctx:
        wpool = ctx.enter_context(tc.tile_pool(name="wpool", bufs=1))
        xpool = ctx.enter_context(tc.tile_pool(name="xpool", bufs=2))
        hpool = ctx.enter_context(tc.tile_pool(name="hpool", bufs=1))
        spool = ctx.enter_context(tc.tile_pool(name="spool", bufs=2))
        opool = ctx.enter_context(tc.tile_pool(name="opool", bufs=2))
        ps12 = ctx.enter_context(tc.tile_pool(name="ps12", bufs=2, space="PSUM"))
        ps3 = ctx.enter_context(tc.tile_pool(name="ps3", bufs=3, space="PSUM"))

        # All weights resident for the whole kernel (bf16: 96 KiB/partition).
        wg_sb = wpool.tile([P, KO, H], BF16)
        wu_sb = wpool.tile([P, KO, H], BF16)
        wd_sb = wpool.tile([P, HO, O], BF16)

        xt_tiles = {}

        def load_xt(tb):
            t = xpool.tile([P, KO, TB], BF16, tag="xt", name=f"xt_sb{tb}")
            for ko in range(KO):
                nc.sync.dma_start(
                    t[:, ko, :],
                    xt[ko * P : (ko + 1) * P, tb * TB : (tb + 1) * TB],
                )
            xt_tiles[tb] = t

        # Startup DMA order (single sync queue, in program order): the first
        # gate chain needs wg chunk 0 + xt block 0, so those go first; wd is
        # only read ~100us in (block-0 down projection), so it loads last.
        # The startup window is DMA-bandwidth-bound — issuing the PE earlier
        # (smaller fragments / parallel queues) just converts the wait into
        # stream gaps and delays the HAM clock-gate warm-up (measured).
        for c in range(NWC):
            csl = slice(c * WCH, (c + 1) * WCH)
            nc.sync.dma_start(
                wg_sb[:, :, csl],
                wg[:, csl].rearrange("(ko p) h -> p ko h", p=P),
            )
            nc.sync.dma_start(
                wu_sb[:, :, csl],
                wu[:, csl].rearrange("(ko p) h -> p ko h", p=P),
            )
            if c == 0:
                load_xt(0)
        for ho in range(HO):
            nc.sync.dma_start(wd_sb[:, ho, :], wd[ho * P : (ho + 1) * P, :])

        for tb in range(NTB):
            if tb + 1 < NTB:
                load_xt(tb + 1)
            xt_sb = xt_tiles.pop(tb)
            hid_sb = hpool.tile([P, HO, TB], BF16, tag="hid")

            for h in range(HO):
                for th in range(NT):
                    tsl = slice(th * MMF, (th + 1) * MMF)
                    gate_ps = ps12.tile([P, MMF], F32, tag="gate")
                    for ko in range(KO):
                        nc.tensor.matmul(
                            gate_ps[:],
                            wg_sb[:, ko, h * P : (h + 1) * P],
                            xt_sb[:, ko, tsl],
                            start=(ko == 0),
                            stop=(ko == KO - 1),
                        )
                    up_ps = ps12.tile([P, MMF], F32, tag="up")
                    for ko in range(KO):
                        nc.tensor.matmul(
                            up_ps[:],
                            wu_sb[:, ko, h * P : (h + 1) * P],
                            xt_sb[:, ko, tsl],
                            start=(ko == 0),
                            stop=(ko == KO - 1),
                        )
                    silu_sb = spool.tile([P, MMF], F32, tag="silu")
                    nc.scalar.activation(silu_sb[:], gate_ps[:], silu_fn)
                    nc.vector.tensor_mul(
                        hid_sb[:, h, tsl], silu_sb[:], up_ps[:]
                    )

            # Down projection for this token block.
            for ti in range(TB // P):
                for oi in range(NO):
                    osl = slice(oi * MMF, (oi + 1) * MMF)
                    out_ps = ps3.tile([P, MMF], F32, tag="outp")
                    for ho in range(HO):
                        nc.tensor.matmul(
                            out_ps[:],
                            hid_sb[:, ho, ti * P : (ti + 1) * P],
                            wd_sb[:, ho, osl],
                            start=(ho == 0),
                            stop=(ho == HO - 1),
                        )
                    # Copy + out-DMA both on the scalar queue (idle during the
                    # down phase): same-queue ordering needs no cross-engine
                    # semaphore, and vector stays free for the SwiGLU muls.
                    ob = opool.tile([P, MMF], F32, tag="ob")
                    rows = slice(tb * TB + ti * P, tb * TB + (ti + 1) * P)
                    if tb == NTB - 1 and ti == TB // P - 1 and oi == NO - 1:
                        # Very last tile: nothing left to overlap with, so
                        # pipeline copy/DMA halves to shorten the drain tail.
                        hf = MMF // 2
                        for q in range(2):
                            qsl = slice(q * hf, (q + 1) * hf)
                            osl_q = slice(oi * MMF + q * hf, oi * MMF + (q + 1) * hf)
                            nc.scalar.activation(
                                ob[:, qsl],
                                out_ps[:, qsl],
                                mybir.ActivationFunctionType.Copy,
                            )
                            nc.scalar.dma_start(out[rows, osl_q], ob[:, qsl])
                    else:
                        nc.scalar.activation(
                            ob[:], out_ps[:], mybir.ActivationFunctionType.Copy
                        )
                        nc.scalar.dma_start(out[rows, osl], ob[:])

    nc.compile()
    return nc


def _get_nc():
    global _CACHED_NC
    if _CACHED_NC is None:
        _CACHED_NC = _build_nc()
    return _CACHED_NC


def _make_in_maps(x, gate_weight, up_weight, down_weight, n):
    import ml_dtypes

    bf16 = ml_dtypes.bfloat16
    in_maps = []
    for g in range(n):
        in_maps.append(
            {
                "xt": np.ascontiguousarray(x[g::n].T).astype(bf16),
                "wg": np.ascontiguousarray(gate_weight[g]).astype(bf16),
                "wu": np.ascontiguousarray(up_weight[g]).astype(bf16),
                "wd": np.ascontiguousarray(down_weight[g]).astype(bf16),
            }
        )
    return in_maps


def _run_spmd(in_maps, **kwargs):
    nc = _get_nc()
    return run_bass_kernel_spmd(nc, in_maps, core_ids=list(range(N_CORES)), **kwargs)


def kernel(x, gate_weight, up_weight, down_weight, num_groups=8):
    n = int(num_groups)
    x = np.asarray(x, dtype=np.float32)
    gate_weight = np.asarray(gate_weight, dtype=np.float32)
    up_weight = np.asarray(up_weight, dtype=np.float32)
    down_weight = np.asarray(down_weight, dtype=np.float32)

    assert n == N_CORES, f"expected {N_CORES} groups, got {n}"
    assert x.shape == (T * N_CORES, K), x.shape
    assert gate_weight.shape == (n, K, H), gate_weight.shape
    assert up_weight.shape == (n, K, H), up_weight.shape
    assert down_weight.shape == (n, H, O), down_weight.shape

    in_maps = _make_in_maps(x, gate_weight, up_weight, down_weight, n)
    res = _run_spmd(in_maps)

    out = np.empty((x.shape[0], O), dtype=np.float32)
    for g in range(n):
        out[g::n] = res.results[g]["out"]
    return out
